# revision 25
# baseline (speedup 1.0000x reference)
"""Trainium2 Bass kernel for DiT attention (nn_DiTAttention_39651138076999).

Sharding: 2-way batch x 4-way head-group over 8 NeuronCores.
Core c handles batch c//4 and heads [4*(c%4) .. 4*(c%4)+3].

Key insight: QK L2-normalization bounds every logit to |q.k|*HD^-0.5 <=
0.125 (Cauchy-Schwarz), so exp(s) = 1 + s to 8e-3 absolute (1.8e-4 final
rel err, measured in f64).  Attention therefore collapses to exact-enough
LINEAR attention, and because the softmax denominator d = S + eps with
|eps| <= 5.7 << S, the division linearizes too:

    out ~= (sum_v + q_hat . KV'') / S,
    KV'' = KV - sum_k_hat (x) (sum_v / S)     # rank-1 correction
    KV   = sum_k (k_hat*scale) [v_k | 1]^T    # [64, 65] per head

(dropped terms <= 1.2e-4 rel).  This removes the S x S score/exp/AV
pipeline, all reciprocals, and the per-query normalize broadcast.

Per-core pipeline (DRAM I/O bf16 + fp8, matmuls bf16/fp8, PSUM f32):
  1. q: dims-major pair tiles ([128, S], 2 heads stacked): fp8 DoubleRow
     projection (4 double-K matmuls, weights pre-scaled x16 on host, the
     1/16 undone in the ACT staging copy -- the L2-norm would kill any
     scale anyway) + RoPE pre-swap trick + L2-normalize.
  2. k: seq-major v-style [seq, head*64]: fp8 DoubleRow projection; RoPE
     in the free dim with strided even/odd adds against stride-0
     head-broadcast tables; L2-norm via ACT Square + DVE reduce and a
     per-partition tensor_scalar; SCALE folded in.  v: bf16 projection
     (fp8 would cost ~1.8% output error), embedded ones column.
  3. KV: per pair one [128, 130] PSUM accumulator (A block rows 0:64
     cols 0:65, B rows 64:128 cols 65:130), 2 matmuls (N=130) per seq
     chunk.  After 16 chunks: copy to SBUF, pull sum_k_hat rows out of
     column 64/129 via tiny PE transposes, apply the rank-1 correction
     with two K=1 outer-product matmuls per pair, re-copy.
  4. attn: per (pair, qq) two [65, 512] matmuls (tile_position row 64
     for head B); ao = (o + sum_v) * (1/S) in one fused tensor_scalar
     per head (sum_v host-exact f32 column); head B to partitions
     64:128 via SBUF->SBUF DMA.
  5. Out-projection: K=128 stationary pair tiles, [128, 1024] staging
     and one row-block DMA per seq chunk; bf16 partials summed on host
     with out_b.
"""
import numpy as np
import ml_dtypes

import concourse.bacc as bacc
import concourse.bass as bass
import concourse.tile as tile
from concourse import mybir
from concourse.bass import broadcast_tensor_aps
from concourse.bass_utils import run_bass_kernel_spmd

B, S, D, H, HD = 2, 2048, 1024, 16, 64
HALF = HD // 2
SCALE = float(HD) ** -0.5
W8SCALE = 16.0
NCORES = 8
P = 128
NSL = 4            # 512-wide slices per 2048
SL = 512
KC = 8             # D // 128 contraction chunks
SC = 16            # S // 128 seq chunks

f32 = mybir.dt.float32
bf16 = mybir.dt.bfloat16
fp8 = mybir.dt.float8e4
DR = mybir.MatmulPerfMode.DoubleRow

_CACHE = {}


def _rope_tables():
    positions = np.arange(S, dtype=np.float32)
    freqs = np.arange(HALF, dtype=np.float32)
    inv_freq = (np.float32(1.0) / (np.float32(10000.0) ** (freqs / np.float32(HALF)))).astype(np.float32)
    theta = positions[:, None] * inv_freq[None, :]          # [S, 32]
    sin = np.sin(theta).astype(np.float32)
    cos = np.cos(theta).astype(np.float32)
    d = np.arange(P)
    f = (d % HD) // 2
    CT = np.ascontiguousarray(cos[:, f].T)                  # [128, S]
    # pre-swap signed sin: even dims +sin, odd dims -sin
    STp = np.ascontiguousarray(
        np.where((d % 2 == 0)[:, None], sin[:, f].T, -sin[:, f].T)).astype(np.float32)
    return CT.astype(ml_dtypes.bfloat16), STp.astype(ml_dtypes.bfloat16)


def _rope_tables_seq():
    """Seq-major single-head tables [128, SC, 64]: value (p, sc, d) for
    seq = sc*128 + p (broadcast across the 4 heads via stride-0 APs)."""
    positions = np.arange(S, dtype=np.float32)
    freqs = np.arange(HALF, dtype=np.float32)
    inv_freq = (np.float32(1.0) / (np.float32(10000.0) ** (freqs / np.float32(HALF)))).astype(np.float32)
    theta = positions[:, None] * inv_freq[None, :]          # [S, 32]
    d = np.arange(HD)
    f = d // 2
    cs = np.cos(theta)[:, f]                                # [S, 64]
    ss = np.sin(theta)[:, f]
    ssp = np.where((d % 2 == 0)[None, :], ss, -ss)          # pre-swap sign
    CTs = np.ascontiguousarray(
        cs.reshape(SC, P, HD).transpose(1, 0, 2).reshape(P, SC * HD))
    STs = np.ascontiguousarray(
        ssp.reshape(SC, P, HD).transpose(1, 0, 2).reshape(P, SC * HD))
    return CTs.astype(ml_dtypes.bfloat16), STs.astype(ml_dtypes.bfloat16)


def _consts():
    CT, STp = _rope_tables()
    CTs, STs = _rope_tables_seq()
    pswap = np.zeros((P, P), dtype=ml_dtypes.bfloat16)
    idx = np.arange(P)
    pswap[idx ^ 1, idx] = 1.0
    bb = np.zeros((P, P), dtype=ml_dtypes.bfloat16)
    bb[0:64, 0:64] = 1.0
    bb[64:128, 64:128] = 1.0
    id64 = np.zeros((P, 64), dtype=ml_dtypes.bfloat16)
    id64[np.arange(P), np.arange(P) % 64] = 1.0
    return CT, STp, CTs, STs, pswap, bb, id64


def _build():
    nc = bacc.Bacc('TRN2')
    x8 = nc.declare_dram_parameter("x8", [P, KC * S], fp8, isOutput=False)
    r8 = nc.declare_dram_parameter("r8", [P, KC * S], fp8, isOutput=False)
    wq8 = nc.declare_dram_parameter("wq8", [P, 2 * KC * P], fp8, isOutput=False)
    wk8 = nc.declare_dram_parameter("wk8", [P, KC * 256], fp8, isOutput=False)
    wv8 = nc.declare_dram_parameter("wv8", [P, KC * 256], fp8, isOutput=False)
    wv8s = nc.declare_dram_parameter("wv8s", [P, KC * 256], fp8, isOutput=False)
    wout = nc.declare_dram_parameter("wout", [P, 2 * D], bf16, isOutput=False)
    ct_d = nc.declare_dram_parameter("ct", [P, S], bf16, isOutput=False)
    st_d = nc.declare_dram_parameter("st", [P, S], bf16, isOutput=False)
    cts_d = nc.declare_dram_parameter("cts", [P, SC * HD], bf16, isOutput=False)
    sts_d = nc.declare_dram_parameter("sts", [P, SC * HD], bf16, isOutput=False)
    pswap_d = nc.declare_dram_parameter("pswap", [P, P], bf16, isOutput=False)
    bb_d = nc.declare_dram_parameter("bb", [P, P], bf16, isOutput=False)
    id64_d = nc.declare_dram_parameter("id64", [P, 64], bf16, isOutput=False)
    sv_d = nc.declare_dram_parameter("sv", [64, 4], f32, isOutput=False)
    svs_d = nc.declare_dram_parameter("svs", [64, 4], f32, isOutput=False)
    svrow_d = nc.declare_dram_parameter("svrow", [1, 4 * 65], bf16,
                                        isOutput=False)
    part = nc.declare_dram_parameter("part", [S, D], bf16, isOutput=True)

    with tile.TileContext(nc) as tc:
        _body(nc, tc, x8, r8, wq8, wk8, wv8, wv8s, wout, ct_d, st_d,
              cts_d, sts_d, pswap_d, bb_d, id64_d, sv_d, svs_d, svrow_d, part)
    nc.compile()
    return nc


def _body(nc, tc, x8, r8, wq8, wk8, wv8, wv8s, wout, ct_d, st_d,
          cts_d, sts_d, pswap_d, bb_d, id64_d, sv_d, svs_d, svrow_d, part):
    from contextlib import ExitStack

    with ExitStack() as ctx:
        persist = ctx.enter_context(tc.tile_pool(name="persist", bufs=1))
        ct_sb = persist.tile([P, S], bf16)
        st_sb = persist.tile([P, S], bf16)
        cts_sb = persist.tile([P, SC, HD], bf16)
        sts_sb = persist.tile([P, SC, HD], bf16)
        pswap_sb = persist.tile([P, P], bf16)
        bb_sb = persist.tile([P, P], bf16)
        id64_sb = persist.tile([P, 64], bf16)
        sv_sb = persist.tile([64, 4], f32)
        svs_sb = persist.tile([64, 4], f32)
        svrow_sb = persist.tile([1, 4, 65], bf16)
        skm_sb = persist.tile([1, 4, 64], bf16)
        x8_sb = persist.tile([P, KC, S], fp8)
        r8_sb = persist.tile([P, KC, S], fp8)
        wq8_sb = persist.tile([P, 2 * KC * P], fp8)
        wk8_sb = persist.tile([P, KC * 256], fp8)
        wv8_sb = persist.tile([P, KC * 256], fp8)
        wv8s_sb = persist.tile([P, KC * 256], fp8)
        wout_sb = persist.tile([P, 2 * D], bf16)         # [128, 2048]

        # v with embedded ones columns: [128, sc(16), head(4), 65] bf16
        v_sb = persist.tile([P, SC, 4, 65], bf16)
        nc.vector.memset(v_sb[:, :, :, 64:65], SCALE)
        # k_hat seq-major: [128, sc(16), head(4), 64] bf16
        kh_sb = persist.tile([P, SC, 4, HD], bf16)

        # rotated+normalized q pair tiles (bf16)
        qhat = [persist.tile([P, S], bf16, tag=f"qhat{i}", name=f"qhat{i}")
                for i in range(2)]
        # KV pair tiles in SBUF: [128, 130]
        kv_sb = [persist.tile([P, 130], bf16, tag=f"kv{i}", name=f"kv{i}")
                 for i in range(2)]
        # packed attention outputs: pair tile [128, S], head B at parts 64:128
        ao = [persist.tile([P, S], bf16, tag=f"ao{i}", name=f"ao{i}")
              for i in range(2)]

        psA = ctx.enter_context(tc.tile_pool(name="psA", bufs=2, space="PSUM"))
        kv_ps_pool = ctx.enter_context(
            tc.tile_pool(name="kvps", bufs=2, space="PSUM"))
        pv_ps = ctx.enter_context(tc.tile_pool(name="pvps", bufs=2, space="PSUM"))
        po_ps = ctx.enter_context(tc.tile_pool(name="pops", bufs=2, space="PSUM"))
        rope_tmp = ctx.enter_context(tc.tile_pool(name="ropetmp", bufs=4))
        krope = ctx.enter_context(tc.tile_pool(name="krope", bufs=4))
        bc_pool = ctx.enter_context(tc.tile_pool(name="bcp", bufs=4))
        out_stage = ctx.enter_context(tc.tile_pool(name="ostg", bufs=4))

        # ---- DMA emission, consumption order.  Startup latency matters:
        # q_slice(0,0) needs wq8 (SP, first) + x8 slice 0 (ACT, first);
        # wk8 rides the otherwise-idle DVE queue; rope tables on the Pool
        # SWDGE queue; everything else follows its first consumer. ----
        nc.sync.dma_start(out=wq8_sb, in_=wq8[:, :])
        nc.scalar.dma_start(
            out=x8_sb[:, :, 0:SL],
            in_=x8.rearrange("p (kc s) -> p kc s", kc=KC)[:, :, 0:SL])
        nc.gpsimd.dma_start(out=wk8_sb, in_=wk8[:, :])
        nc.sync.dma_start(
            out=r8_sb[:, :, 0:SL],
            in_=r8.rearrange("p (kc s) -> p kc s", kc=KC)[:, :, 0:SL])
        nc.scalar.dma_start(out=wv8_sb, in_=wv8[:, :])
        nc.sync.dma_start(out=wv8s_sb, in_=wv8s[:, :])
        nc.gpsimd.dma_start(out=st_sb, in_=st_d[:, :])
        nc.gpsimd.dma_start(out=ct_sb, in_=ct_d[:, :])
        nc.gpsimd.dma_start(out=sts_sb,
                            in_=sts_d.rearrange("p (sc d) -> p sc d", sc=SC))
        nc.gpsimd.dma_start(out=cts_sb,
                            in_=cts_d.rearrange("p (sc d) -> p sc d", sc=SC))
        nc.sync.dma_start(out=pswap_sb, in_=pswap_d[:, :])
        nc.sync.dma_start(out=bb_sb, in_=bb_d[:, :])
        for c4 in range(1, NSL):
            cs = slice(c4 * SL, (c4 + 1) * SL)
            nc.scalar.dma_start(
                out=x8_sb[:, :, cs],
                in_=x8.rearrange("p (kc s) -> p kc s", kc=KC)[:, :, cs])
            nc.sync.dma_start(
                out=r8_sb[:, :, cs],
                in_=r8.rearrange("p (kc s) -> p kc s", kc=KC)[:, :, cs])
        nc.scalar.dma_start(out=wout_sb, in_=wout[:, :])
        nc.sync.dma_start(out=id64_sb, in_=id64_d[:, :])
        nc.sync.dma_start(out=sv_sb, in_=sv_d[:, :])
        nc.sync.dma_start(out=svs_sb, in_=svs_d[:, :])
        nc.sync.dma_start(
            out=svrow_sb,
            in_=svrow_d.rearrange("o (h e) -> o h e", h=4))

        # persistent KV PSUM accumulators (one bank per pair)
        kv_ps = [kv_ps_pool.tile([P, 130], f32, tag="kvps",
                                 name=f"kvps{i}") for i in range(2)]

        Copy = mybir.ActivationFunctionType.Copy
        Identity = mybir.ActivationFunctionType.Identity
        AbsRsqrt = mybir.ActivationFunctionType.Abs_reciprocal_sqrt
        Square = mybir.ActivationFunctionType.Square
        Sqrt = mybir.ActivationFunctionType.Sqrt

        def q_slice(ti, sl):
            """Project q pair tile ti for seq slice sl (fp8 DoubleRow),
            rope + normalize.  The swap and sum-of-squares matmuls write
            back over the qkp PSUM region (WAR-serialized by Tile)."""
            sls = slice(sl * SL, (sl + 1) * SL)
            qkp = psA.tile([P, SL], f32, tag="psA", name="qkp")
            for j in range(KC // 2):
                nc.tensor.matmul(
                    qkp,
                    wq8_sb[:, (ti * KC + 2 * j) * P:(ti * KC + 2 * j + 2) * P]
                    .rearrange("p (two m) -> p two m", two=2),
                    x8_sb[:, 2 * j:2 * j + 2, sls],
                    start=(j == 0), stop=(j == KC // 2 - 1), perf_mode=DR)
            # stage to SBUF on ACT (scale undoes the x16 weight prescale)
            raw = rope_tmp.tile([P, SL], bf16, tag="raw", name="raw")
            nc.scalar.activation(raw, qkp, Copy, scale=1.0 / W8SCALE)
            m2p = rope_tmp.tile([P, SL], bf16, tag="m2p", name="m2p")
            nc.gpsimd.tensor_mul(m2p, raw, st_sb[:, sls])
            m1 = rope_tmp.tile([P, SL], bf16, tag="m1", name="m1")
            nc.vector.tensor_mul(m1, raw, ct_sb[:, sls])
            nc.tensor.matmul(qkp, pswap_sb, m2p, start=True, stop=True,
                             skip_group_check=True)
            rot = rope_tmp.tile([P, SL], bf16, tag="rot", name="rot")
            nc.vector.tensor_add(rot, m1, qkp)
            sq = rope_tmp.tile([P, SL], bf16, tag="sq", name="sq")
            nc.scalar.activation(sq, rot, Square)
            nc.tensor.matmul(qkp, bb_sb, sq, start=True, stop=True,
                             skip_group_check=True)
            # rsqrt via the ACT table (q_hat is scale-invariant, so table
            # error only perturbs the logit scale by <=0.1%)
            rsq = rope_tmp.tile([P, SL], bf16, tag="rsq", name="rsq")
            nc.scalar.activation(rsq, qkp, AbsRsqrt)
            nc.vector.tensor_mul(qhat[ti][:, sls], rot, rsq)

        def kv_chunk(sc):
            """Project k (fp8 DR) and v (bf16) for seq chunk sc (v-style
            [seq, 4*64]), rope + normalize k, accumulate both pairs' KV."""
            kp = pv_ps.tile([P, 256], f32, tag="pvps", name="kp")
            for j in range(KC // 2):
                nc.tensor.matmul(
                    kp,
                    x8_sb[:, 2 * j:2 * j + 2, sc * P:(sc + 1) * P],
                    wk8_sb[:, 2 * j * 256:(2 * j + 2) * 256]
                    .rearrange("p (two m) -> p two m", two=2),
                    start=(j == 0), stop=(j == KC // 2 - 1), perf_mode=DR)
            vp = pv_ps.tile([P, 256], f32, tag="pvps", name="vp")
            for j in range(KC // 2):
                nc.tensor.matmul(
                    vp,
                    x8_sb[:, 2 * j:2 * j + 2, sc * P:(sc + 1) * P],
                    wv8_sb[:, 2 * j * 256:(2 * j + 2) * 256]
                    .rearrange("p (two m) -> p two m", two=2),
                    start=(j == 0), stop=False, perf_mode=DR)
            for j in range(KC // 2):
                nc.tensor.matmul(
                    vp,
                    r8_sb[:, 2 * j:2 * j + 2, sc * P:(sc + 1) * P],
                    wv8s_sb[:, 2 * j * 256:(2 * j + 2) * 256]
                    .rearrange("p (two m) -> p two m", two=2),
                    start=False, stop=(j == KC // 2 - 1), perf_mode=DR)
            # SCALE/16 rides on the v side (vp holds 16*v; k_hat is
            # normalization-invariant so KV picks up exactly SCALE)
            if sc % 2 == 0:
                nc.vector.tensor_scalar(
                    v_sb[:, sc, :, 0:64],
                    vp.rearrange("p (h d) -> p h d", h=4),
                    SCALE / 16.0, None, mybir.AluOpType.mult)
            else:
                nc.scalar.activation(
                    v_sb[:, sc, :, 0:64],
                    vp.rearrange("p (h d) -> p h d", h=4),
                    Copy, scale=SCALE / 16.0)
            # k rope in the free dim; stage kp to SBUF on ACT (undo x16)
            kraw = krope.tile([P, 4, HD], bf16, tag="kraw", name="kraw")
            nc.scalar.activation(kraw.rearrange("p h d -> p (h d)"), kp, Copy,
                                 scale=1.0 / W8SCALE)
            m2 = krope.tile([P, 4, HD], bf16, tag="km2", name="km2")
            in0b, in1b = broadcast_tensor_aps(kraw[:, :, :],
                                              sts_sb[:, sc:sc + 1, :])
            nc.gpsimd.tensor_tensor(m2, in0b, in1b, mybir.AluOpType.mult)
            m1 = krope.tile([P, 4, HD], bf16, tag="km1", name="km1")
            in0c, in1c = broadcast_tensor_aps(kraw[:, :, :],
                                              cts_sb[:, sc:sc + 1, :])
            nc.vector.tensor_tensor(m1, in0c, in1c, mybir.AluOpType.mult)
            rot = krope.tile([P, 4, 32, 2], bf16, tag="krot", name="krot")
            m1v = m1.rearrange("p h (d two) -> p h d two", two=2)
            m2v = m2.rearrange("p h (d two) -> p h d two", two=2)
            # rot_even = m1_even + m2_odd ; rot_odd = m1_odd + m2_even
            nc.vector.tensor_add(rot[:, :, :, 0], m1v[:, :, :, 0],
                                 m2v[:, :, :, 1])
            nc.gpsimd.tensor_add(rot[:, :, :, 1], m1v[:, :, :, 1],
                                 m2v[:, :, :, 0])
            sq = krope.tile([P, 256], bf16, tag="ksq", name="ksq")
            rotf = rot.rearrange("p h d two -> p (h d two)")
            nc.gpsimd.tensor_mul(sq, rotf, rotf)
            ssq = krope.tile([P, 4], bf16, tag="kssq", name="kssq")
            with nc.allow_low_precision(reason="k ssq ~64, bf16 adds 0.2% to "
                                        "an 0.4%-noise quantity"):
                nc.vector.tensor_reduce(
                    ssq, sq.rearrange("p (h d) -> p h d", h=4),
                    mybir.AxisListType.X, mybir.AluOpType.add)
            rsq = krope.tile([P, 4, 1], bf16, tag="krsq", name="krsq")
            nc.scalar.activation(rsq.rearrange("p h o -> p (h o)"), ssq,
                                 AbsRsqrt)
            in0n, in1n = broadcast_tensor_aps(
                rot.rearrange("p h d two -> p h (d two)"), rsq[:, :, :])
            eng = nc.vector if sc % 2 == 0 else nc.gpsimd
            eng.tensor_tensor(kh_sb[:, sc, :, :], in0n, in1n,
                              mybir.AluOpType.mult)
            for pi in range(2):
                nc.tensor.matmul(
                    kv_ps[pi],
                    kh_sb[:, sc, 2 * pi:2 * pi + 2, :].rearrange(
                        "p h d -> p (h d)"),
                    v_sb[:, sc, 2 * pi:2 * pi + 2, :].rearrange(
                        "p h e -> p (h e)"),
                    start=(sc == 0), stop=False,
                    skip_group_check=True)

        def kv_finish():
            # pass 1: KV to SBUF
            for pi in range(2):
                nc.vector.tensor_copy(kv_sb[pi], kv_ps[pi])
            # pull sum_k_hat rows out of columns 64 / 129 via PE transpose,
            # scale by -1/S, then rank-1 update KV'' = KV - sk (x) svrow/S.
            # svrow's 65th entry is S, which zeroes column 64 (unused after).
            for i, (pr, cr) in enumerate(((slice(0, 64), 64),
                                          (slice(64, 128), 129),
                                          (slice(0, 64), 64),
                                          (slice(64, 128), 129))):
                pi = i // 2
                tp = po_ps.tile([1, 64], bf16, tag="pops", name="tp")
                nc.tensor.matmul(tp, kv_sb[pi][pr, cr:cr + 1],
                                 id64_sb[pr, :], is_transpose=True)
                nc.vector.tensor_scalar(skm_sb[:, i, :], tp, -1.0 / S, None,
                                        mybir.AluOpType.mult)
            for pi in range(2):
                nc.tensor.matmul(
                    kv_ps[pi][0:64, 0:65], skm_sb[:, 2 * pi, :],
                    svrow_sb[:, 2 * pi, :],
                    start=False, stop=False, skip_group_check=True)
                nc.tensor.matmul(
                    kv_ps[pi][64:128, 65:130], skm_sb[:, 2 * pi + 1, :],
                    svrow_sb[:, 2 * pi + 1, :],
                    start=False, stop=True, skip_group_check=True,
                    tile_position=(0, 64))
            # pass 2: corrected KV to SBUF
            for pi in range(2):
                nc.vector.tensor_copy(kv_sb[pi], kv_ps[pi])

        def attn_qq(pi, qq):
            qqs = slice(qq * SL, (qq + 1) * SL)
            o_a = po_ps.tile([65, SL], f32, tag="pops", name="o_a")
            o_b = po_ps.tile([65, SL], f32, tag="pops", name="o_b")
            nc.tensor.matmul(o_a, kv_sb[pi][0:64, 0:65],
                             qhat[pi][0:64, qqs], start=True, stop=True)
            nc.tensor.matmul(o_b, kv_sb[pi][64:128, 65:130],
                             qhat[pi][64:128, qqs], start=True, stop=True,
                             tile_position=(64, 0))
            # ao = (o + sum_v) / S: head A fused on DVE, head B on ACT
            # (Identity with per-partition bias = sum_v/S); B staged then
            # DMA'd to partitions 64:128
            nc.vector.tensor_scalar(ao[pi][0:64, qqs], o_a[0:64, :],
                                    sv_sb[:, 2 * pi:2 * pi + 1], 1.0 / S,
                                    mybir.AluOpType.add, mybir.AluOpType.mult)
            tm_b = bc_pool.tile([64, SL], bf16, tag="tmb", name="tm_b")
            nc.scalar.activation(tm_b, o_b[0:64, :], Identity,
                                 bias=svs_sb[:, 2 * pi + 1:2 * pi + 2],
                                 scale=1.0 / S)
            nc.sync.dma_start(out=ao[pi][64:128, qqs], in_=tm_b)

        def outproj_qq(qq):
            for sc in range(4 * qq, 4 * qq + 4):
                stg = out_stage.tile([P, 2 * SL], bf16, tag="ostg", name="stg")
                for osl in range(2):
                    if osl == 0:
                        op = kv_ps_pool.tile([P, SL], f32, tag="kvps",
                                             name="op")
                    else:
                        op = psA.tile([P, SL], f32, tag="psA", name="op")
                    nc.tensor.matmul(
                        op, ao[0][:, sc * P:(sc + 1) * P],
                        wout_sb[:, osl * SL:(osl + 1) * SL],
                        start=True, stop=False)
                    nc.tensor.matmul(
                        op, ao[1][:, sc * P:(sc + 1) * P],
                        wout_sb[:, D + osl * SL:D + (osl + 1) * SL],
                        start=False, stop=True)
                    if osl == 0:
                        nc.scalar.activation(stg[:, 0:SL], op, Copy)
                    else:
                        nc.vector.tensor_copy(stg[:, SL:2 * SL], op)
                nc.gpsimd.dma_start(out=part[sc * P:(sc + 1) * P, :],
                                    in_=stg)

        # ---- emission order = scheduling priority ----
        for ql in range(NSL):
            q_slice(0, ql)
            kv_chunk(4 * ql + 0)
            kv_chunk(4 * ql + 1)
            q_slice(1, ql)
            kv_chunk(4 * ql + 2)
            kv_chunk(4 * ql + 3)
        kv_finish()
        attn_qq(0, 0)
        attn_qq(1, 0)
        attn_qq(0, 1)
        outproj_qq(0)
        attn_qq(1, 1)
        attn_qq(0, 2)
        outproj_qq(1)
        attn_qq(1, 2)
        attn_qq(0, 3)
        outproj_qq(2)
        attn_qq(1, 3)
        outproj_qq(3)


def _host_prep(tokens, qkv_w, qkv_b, out_w):
    """Build the 8 per-core input maps."""
    CT, STp, CTs, STs, pswap, bb, id64 = _consts()
    in_maps = []
    for core in range(NCORES):
        b = core // 4
        g = core % 4
        heads = [4 * g + i for i in range(4)]
        xTf = np.ascontiguousarray(tokens[b].T)
        xkc = np.ascontiguousarray(
            xTf.reshape(KC, P, S).transpose(1, 0, 2).reshape(P, KC * S))
        x8 = xkc.astype(ml_dtypes.float8_e4m3)
        r8 = (16.0 * (xkc - x8.astype(np.float32))).astype(
            ml_dtypes.float8_e4m3)

        def wq_tile(pair):
            rows = np.r_[heads[2 * pair] * HD:heads[2 * pair] * HD + HD,
                         heads[2 * pair + 1] * HD:
                         heads[2 * pair + 1] * HD + HD]
            Wt = qkv_w[rows] * W8SCALE                               # [128, D]
            return np.ascontiguousarray(Wt.T).reshape(KC, P, P).transpose(1, 0, 2).reshape(P, KC * P)

        wq8 = np.ascontiguousarray(
            np.concatenate([wq_tile(0), wq_tile(1)], axis=1)).astype(
                ml_dtypes.float8_e4m3)

        def vstyle(base, scale):
            rows = np.r_[tuple(np.arange(base + h * HD, base + (h + 1) * HD)
                               for h in heads)]
            WT = np.ascontiguousarray(qkv_w[rows].T) * scale         # [D, 256]
            return WT.reshape(KC, P, 256).transpose(1, 0, 2).reshape(
                P, KC * 256)

        wk8 = vstyle(D, W8SCALE).astype(ml_dtypes.float8_e4m3)
        wv8 = vstyle(2 * D, 16.0).astype(ml_dtypes.float8_e4m3)
        wv8s = vstyle(2 * D, 1.0).astype(ml_dtypes.float8_e4m3)

        wout_blocks = []
        for pair in range(2):
            wcols = np.r_[tuple(np.arange(h * HD, (h + 1) * HD)
                                for h in heads[2 * pair:2 * pair + 2])]
            wout_blocks.append(np.ascontiguousarray(out_w[:, wcols].T))  # [128, D]
        wout_h = np.ascontiguousarray(
            np.concatenate(wout_blocks, axis=1)).astype(ml_dtypes.bfloat16)

        # host-exact sum_k v per head: (sum_s x) @ Wv^T in f64
        xsum = tokens[b].astype(np.float64).sum(axis=0)              # [D]
        sv = np.empty((64, 4), dtype=np.float32)
        for i, h in enumerate(heads):
            Wvh = qkv_w[2 * D + h * HD:2 * D + (h + 1) * HD].astype(np.float64)
            sv[:, i] = (Wvh @ xsum).astype(np.float32)
        svs = (sv / np.float32(S)).astype(np.float32)
        svrow = np.zeros((1, 4 * 65), dtype=np.float32)
        for i in range(4):
            svrow[0, i * 65:i * 65 + 64] = sv[:, i]
            svrow[0, i * 65 + 64] = float(S)

        in_maps.append({
            "x8": x8, "r8": r8, "wq8": wq8, "wk8": np.ascontiguousarray(wk8),
            "wv8": np.ascontiguousarray(wv8),
            "wv8s": np.ascontiguousarray(wv8s), "wout": wout_h,
            "ct": CT, "st": STp, "cts": CTs, "sts": STs,
            "pswap": pswap, "bb": bb, "id64": id64, "sv": sv, "svs": svs,
            "svrow": svrow.astype(ml_dtypes.bfloat16),
        })
    return in_maps


def kernel(tokens, qkv_w, qkv_b, out_w, out_b, _trace=False, _tmpdir=None):
    tokens = np.asarray(tokens, dtype=np.float32)
    qkv_w = np.asarray(qkv_w, dtype=np.float32)
    qkv_b = np.asarray(qkv_b, dtype=np.float32)
    out_w = np.asarray(out_w, dtype=np.float32)
    out_b = np.asarray(out_b, dtype=np.float32)

    if np.any(qkv_b):
        raise NotImplementedError(
            "kernel compiled for qkv_b == 0 (spec fill: zeros)")
    if "nc" not in _CACHE:
        _CACHE["nc"] = _build()
    nc = _CACHE["nc"]

    in_maps = _host_prep(tokens, qkv_w, qkv_b, out_w)
    res = run_bass_kernel_spmd(nc, in_maps, list(range(NCORES)),
                               trace=_trace, tmpdir=_tmpdir)
    out = np.zeros((B, S, D), dtype=np.float32)
    for core in range(NCORES):
        out[core // 4] += res.results[core]["part"].astype(np.float32)
    out += out_b[None, None, :]
    if _trace:
        return out, res
    return out


# revision 29
# speedup vs baseline: 1.0943x; 1.0943x over previous
"""Trainium2 Bass kernel for DiT attention (nn_DiTAttention_39651138076999).

Sharding: 2-way batch x 4-way head-group over 8 NeuronCores.
Core c handles batch c//4 and heads [4*(c%4) .. 4*(c%4)+3].

Key insight: QK L2-normalization bounds every logit to |q.k|*HD^-0.5 <=
0.125 (Cauchy-Schwarz), so exp(s) = 1 + s to 8e-3 absolute (1.8e-4 final
rel err, measured in f64).  Attention therefore collapses to exact-enough
LINEAR attention, and because the softmax denominator d = S + eps with
|eps| <= 5.7 << S, the division linearizes too:

    out ~= (sum_v + q_hat . KV'') / S,
    KV'' = KV - sum_k_hat (x) (sum_v / S)     # rank-1 correction
    KV   = sum_k (k_hat*scale) [v_k | 1]^T    # [64, 65] per head

(dropped terms <= 1.2e-4 rel).  This removes the S x S score/exp/AV
pipeline, all reciprocals, and the per-query normalize broadcast.

Per-core pipeline (DRAM I/O bf16 + fp8, matmuls bf16/fp8, PSUM f32):
  1. q: dims-major pair tiles ([128, S], 2 heads stacked): fp8 DoubleRow
     projection (4 double-K matmuls, weights pre-scaled x16 on host, the
     1/16 undone in the ACT staging copy -- the L2-norm would kill any
     scale anyway) + RoPE pre-swap trick + L2-normalize.
  2. k: seq-major v-style [seq, head*64]: fp8 DoubleRow projection; RoPE
     in the free dim with strided even/odd adds against stride-0
     head-broadcast tables; L2-norm via ACT Square + DVE reduce and a
     per-partition tensor_scalar; SCALE folded in.  v: bf16 projection
     (fp8 would cost ~1.8% output error), embedded ones column.
  3. KV: per pair one [128, 130] PSUM accumulator (A block rows 0:64
     cols 0:65, B rows 64:128 cols 65:130), 2 matmuls (N=130) per seq
     chunk.  After 16 chunks: copy to SBUF, pull sum_k_hat rows out of
     column 64/129 via tiny PE transposes, apply the rank-1 correction
     with two K=1 outer-product matmuls per pair, re-copy.
  4. attn: per (pair, qq) two [65, 512] matmuls (tile_position row 64
     for head B); ao = (o + sum_v) * (1/S) in one fused tensor_scalar
     per head (sum_v host-exact f32 column); head B to partitions
     64:128 via SBUF->SBUF DMA.
  5. Out-projection: K=128 stationary pair tiles, [128, 1024] staging
     and one row-block DMA per seq chunk; bf16 partials summed on host
     with out_b.
"""
import numpy as np
import ml_dtypes

import concourse.bacc as bacc
import concourse.bass as bass
import concourse.tile as tile
from concourse import mybir
from concourse.bass import broadcast_tensor_aps
from concourse.bass_utils import run_bass_kernel_spmd

B, S, D, H, HD = 2, 2048, 1024, 16, 64
HALF = HD // 2
SCALE = float(HD) ** -0.5
W8SCALE = 16.0
NCORES = 8
P = 128
NSL = 4            # 512-wide slices per 2048
SL = 512
KC = 8             # D // 128 contraction chunks
SC = 16            # S // 128 seq chunks

f32 = mybir.dt.float32
bf16 = mybir.dt.bfloat16
fp8 = mybir.dt.float8e4
DR = mybir.MatmulPerfMode.DoubleRow

_CACHE = {}


def _rope_tables():
    positions = np.arange(S, dtype=np.float32)
    freqs = np.arange(HALF, dtype=np.float32)
    inv_freq = (np.float32(1.0) / (np.float32(10000.0) ** (freqs / np.float32(HALF)))).astype(np.float32)
    theta = positions[:, None] * inv_freq[None, :]          # [S, 32]
    sin = np.sin(theta).astype(np.float32)
    cos = np.cos(theta).astype(np.float32)
    d = np.arange(P)
    f = (d % HD) // 2
    CT = np.ascontiguousarray(cos[:, f].T)                  # [128, S]
    # pre-swap signed sin: even dims +sin, odd dims -sin
    STp = np.ascontiguousarray(
        np.where((d % 2 == 0)[:, None], sin[:, f].T, -sin[:, f].T)).astype(np.float32)
    return CT.astype(ml_dtypes.bfloat16), STp.astype(ml_dtypes.bfloat16)


def _rope_tables_seq():
    """Seq-major single-head tables [128, SC, 64]: value (p, sc, d) for
    seq = sc*128 + p (broadcast across the 4 heads via stride-0 APs)."""
    positions = np.arange(S, dtype=np.float32)
    freqs = np.arange(HALF, dtype=np.float32)
    inv_freq = (np.float32(1.0) / (np.float32(10000.0) ** (freqs / np.float32(HALF)))).astype(np.float32)
    theta = positions[:, None] * inv_freq[None, :]          # [S, 32]
    d = np.arange(HD)
    f = d // 2
    cs = np.cos(theta)[:, f]                                # [S, 64]
    ss = np.sin(theta)[:, f]
    ssp = np.where((d % 2 == 0)[None, :], ss, -ss)          # pre-swap sign
    CTs = np.ascontiguousarray(
        cs.reshape(SC, P, HD).transpose(1, 0, 2).reshape(P, SC * HD))
    STs = np.ascontiguousarray(
        ssp.reshape(SC, P, HD).transpose(1, 0, 2).reshape(P, SC * HD))
    return CTs.astype(ml_dtypes.bfloat16), STs.astype(ml_dtypes.bfloat16)


def _consts():
    CT, STp = _rope_tables()
    CTs, STs = _rope_tables_seq()
    pswap = np.zeros((P, P), dtype=ml_dtypes.bfloat16)
    idx = np.arange(P)
    pswap[idx ^ 1, idx] = 1.0
    bb = np.zeros((P, P), dtype=ml_dtypes.bfloat16)
    bb[0:64, 0:64] = 1.0
    bb[64:128, 64:128] = 1.0
    id64 = np.zeros((P, 64), dtype=ml_dtypes.bfloat16)
    id64[np.arange(P), np.arange(P) % 64] = 1.0
    return CT, STp, CTs, STs, pswap, bb, id64


def _build():
    nc = bacc.Bacc('TRN2')
    x8 = nc.declare_dram_parameter("x8", [P, KC * S], fp8, isOutput=False)
    r8 = nc.declare_dram_parameter("r8", [P, KC * S], fp8, isOutput=False)
    wq8 = nc.declare_dram_parameter("wq8", [P, 2 * KC * P], fp8, isOutput=False)
    wk8 = nc.declare_dram_parameter("wk8", [P, KC * 256], fp8, isOutput=False)
    wv8 = nc.declare_dram_parameter("wv8", [P, KC * 256], fp8, isOutput=False)
    wv8s = nc.declare_dram_parameter("wv8s", [P, KC * 256], fp8, isOutput=False)
    wout = nc.declare_dram_parameter("wout", [P, 2 * D], bf16, isOutput=False)
    ct_d = nc.declare_dram_parameter("ct", [P, S], bf16, isOutput=False)
    st_d = nc.declare_dram_parameter("st", [P, S], bf16, isOutput=False)
    cts_d = nc.declare_dram_parameter("cts", [P, SC * HD], bf16, isOutput=False)
    sts_d = nc.declare_dram_parameter("sts", [P, SC * HD], bf16, isOutput=False)
    pswap_d = nc.declare_dram_parameter("pswap", [P, P], bf16, isOutput=False)
    bb_d = nc.declare_dram_parameter("bb", [P, P], bf16, isOutput=False)
    id64_d = nc.declare_dram_parameter("id64", [P, 64], bf16, isOutput=False)
    sv_d = nc.declare_dram_parameter("sv", [64, 4], f32, isOutput=False)
    svs_d = nc.declare_dram_parameter("svs", [64, 4], f32, isOutput=False)
    svrow_d = nc.declare_dram_parameter("svrow", [1, 4 * 65], bf16,
                                        isOutput=False)
    part = nc.declare_dram_parameter("part", [S, D], bf16, isOutput=True)

    with tile.TileContext(nc) as tc:
        _body(nc, tc, x8, r8, wq8, wk8, wv8, wv8s, wout, ct_d, st_d,
              cts_d, sts_d, pswap_d, bb_d, id64_d, sv_d, svs_d, svrow_d, part)
    nc.compile()
    return nc


def _body(nc, tc, x8, r8, wq8, wk8, wv8, wv8s, wout, ct_d, st_d,
          cts_d, sts_d, pswap_d, bb_d, id64_d, sv_d, svs_d, svrow_d, part):
    from contextlib import ExitStack

    with ExitStack() as ctx:
        persist = ctx.enter_context(tc.tile_pool(name="persist", bufs=1))
        ct_sb = persist.tile([P, S], bf16)
        st_sb = persist.tile([P, S], bf16)
        cts_sb = persist.tile([P, SC, HD], bf16)
        sts_sb = persist.tile([P, SC, HD], bf16)
        pswap_sb = persist.tile([P, P], bf16)
        bb_sb = persist.tile([P, P], bf16)
        id64_sb = persist.tile([P, 64], bf16)
        sv_sb = persist.tile([64, 4], f32)
        svs_sb = persist.tile([64, 4], f32)
        svrow_sb = persist.tile([1, 4, 65], bf16)
        skm_sb = persist.tile([1, 4, 64], bf16)
        x8_sb = persist.tile([P, KC, S], fp8)
        r8_sb = persist.tile([P, KC, S], fp8)
        wq8_sb = persist.tile([P, 2 * KC * P], fp8)
        wk8_sb = persist.tile([P, KC * 256], fp8)
        wv8_sb = persist.tile([P, KC * 256], fp8)
        wv8s_sb = persist.tile([P, KC * 256], fp8)
        wout_sb = persist.tile([P, 2 * D], bf16)         # [128, 2048]

        # v with embedded ones columns: [128, sc(16), head(4), 65] bf16
        v_sb = persist.tile([P, SC, 4, 65], bf16)
        nc.vector.memset(v_sb[:, :, :, 64:65], SCALE)
        # k_hat seq-major: [128, sc(16), head(4), 64] bf16
        kh_sb = persist.tile([P, SC, 4, HD], bf16)

        # rotated+normalized q pair tiles (bf16)
        qhat = [persist.tile([P, S], bf16, tag=f"qhat{i}", name=f"qhat{i}")
                for i in range(2)]
        # KV pair tiles in SBUF: [128, 130]
        kv_sb = [persist.tile([P, 130], bf16, tag=f"kv{i}", name=f"kv{i}")
                 for i in range(2)]
        # packed attention outputs: pair tile [128, S], head B at parts 64:128
        ao = [persist.tile([P, S], bf16, tag=f"ao{i}", name=f"ao{i}")
              for i in range(2)]

        psA = ctx.enter_context(tc.tile_pool(name="psA", bufs=2, space="PSUM"))
        kv_ps_pool = ctx.enter_context(
            tc.tile_pool(name="kvps", bufs=2, space="PSUM"))
        pv_ps = ctx.enter_context(tc.tile_pool(name="pvps", bufs=2, space="PSUM"))
        po_ps = ctx.enter_context(tc.tile_pool(name="pops", bufs=2, space="PSUM"))
        rope_tmp = ctx.enter_context(tc.tile_pool(name="ropetmp", bufs=4))
        krope = ctx.enter_context(tc.tile_pool(name="krope", bufs=4))
        bc_pool = ctx.enter_context(tc.tile_pool(name="bcp", bufs=4))
        out_stage = ctx.enter_context(tc.tile_pool(name="ostg", bufs=4))

        # ---- DMA emission, consumption order.  Startup latency matters:
        # q_slice(0,0) needs wq8 (SP, first) + x8 slice 0 (ACT, first);
        # wk8 rides the otherwise-idle DVE queue; rope tables on the Pool
        # SWDGE queue; everything else follows its first consumer. ----
        nc.sync.dma_start(out=wq8_sb, in_=wq8[:, :])
        nc.scalar.dma_start(
            out=x8_sb[:, :, 0:SL],
            in_=x8.rearrange("p (kc s) -> p kc s", kc=KC)[:, :, 0:SL])
        nc.gpsimd.dma_start(out=wk8_sb, in_=wk8[:, :])
        nc.sync.dma_start(
            out=r8_sb[:, :, 0:SL],
            in_=r8.rearrange("p (kc s) -> p kc s", kc=KC)[:, :, 0:SL])
        nc.scalar.dma_start(out=wv8_sb, in_=wv8[:, :])
        nc.sync.dma_start(out=wv8s_sb, in_=wv8s[:, :])
        nc.gpsimd.dma_start(out=st_sb, in_=st_d[:, :])
        nc.gpsimd.dma_start(out=ct_sb, in_=ct_d[:, :])
        nc.gpsimd.dma_start(out=sts_sb,
                            in_=sts_d.rearrange("p (sc d) -> p sc d", sc=SC))
        nc.gpsimd.dma_start(out=cts_sb,
                            in_=cts_d.rearrange("p (sc d) -> p sc d", sc=SC))
        nc.sync.dma_start(out=pswap_sb, in_=pswap_d[:, :])
        nc.sync.dma_start(out=bb_sb, in_=bb_d[:, :])
        for c4 in range(1, NSL):
            cs = slice(c4 * SL, (c4 + 1) * SL)
            nc.scalar.dma_start(
                out=x8_sb[:, :, cs],
                in_=x8.rearrange("p (kc s) -> p kc s", kc=KC)[:, :, cs])
            nc.sync.dma_start(
                out=r8_sb[:, :, cs],
                in_=r8.rearrange("p (kc s) -> p kc s", kc=KC)[:, :, cs])
        nc.scalar.dma_start(out=wout_sb, in_=wout[:, :])
        nc.sync.dma_start(out=id64_sb, in_=id64_d[:, :])
        nc.sync.dma_start(out=sv_sb, in_=sv_d[:, :])
        nc.sync.dma_start(out=svs_sb, in_=svs_d[:, :])
        nc.sync.dma_start(
            out=svrow_sb,
            in_=svrow_d.rearrange("o (h e) -> o h e", h=4))

        # persistent KV PSUM accumulators (one bank per pair)
        kv_ps = [kv_ps_pool.tile([P, 130], f32, tag="kvps",
                                 name=f"kvps{i}") for i in range(2)]

        Copy = mybir.ActivationFunctionType.Copy
        Identity = mybir.ActivationFunctionType.Identity
        AbsRsqrt = mybir.ActivationFunctionType.Abs_reciprocal_sqrt
        Square = mybir.ActivationFunctionType.Square
        Sqrt = mybir.ActivationFunctionType.Sqrt

        def q_slice(ti, sl):
            """Project q pair tile ti for seq slice sl (fp8 DoubleRow),
            rope + normalize.  The swap and sum-of-squares matmuls write
            back over the qkp PSUM region (WAR-serialized by Tile)."""
            sls = slice(sl * SL, (sl + 1) * SL)
            qkp = psA.tile([P, SL], f32, tag="psA", name="qkp")
            for j in range(KC // 2):
                nc.tensor.matmul(
                    qkp,
                    wq8_sb[:, (ti * KC + 2 * j) * P:(ti * KC + 2 * j + 2) * P]
                    .rearrange("p (two m) -> p two m", two=2),
                    x8_sb[:, 2 * j:2 * j + 2, sls],
                    start=(j == 0), stop=(j == KC // 2 - 1), perf_mode=DR)
            # stage to SBUF on ACT (scale undoes the x16 weight prescale)
            raw = rope_tmp.tile([P, SL], bf16, tag="raw", name="raw")
            nc.scalar.activation(raw, qkp, Copy, scale=1.0 / W8SCALE)
            m2p = rope_tmp.tile([P, SL], bf16, tag="m2p", name="m2p")
            nc.gpsimd.tensor_mul(m2p, raw, st_sb[:, sls])
            m1 = rope_tmp.tile([P, SL], bf16, tag="m1", name="m1")
            nc.vector.tensor_mul(m1, raw, ct_sb[:, sls])
            nc.tensor.matmul(qkp, pswap_sb, m2p, start=True, stop=True,
                             skip_group_check=True)
            rot = rope_tmp.tile([P, SL], bf16, tag="rot", name="rot")
            nc.vector.tensor_add(rot, m1, qkp)
            sq = rope_tmp.tile([P, SL], bf16, tag="sq", name="sq")
            nc.scalar.activation(sq, rot, Square)
            nc.tensor.matmul(qkp, bb_sb, sq, start=True, stop=True,
                             skip_group_check=True)
            # rsqrt via the ACT table (q_hat is scale-invariant, so table
            # error only perturbs the logit scale by <=0.1%)
            rsq = rope_tmp.tile([P, SL], bf16, tag="rsq", name="rsq")
            nc.scalar.activation(rsq, qkp, AbsRsqrt)
            nc.vector.tensor_mul(qhat[ti][:, sls], rot, rsq)

        def kv_chunk2(sc2):
            """Project k (fp8 DR) and v (split-precision fp8 DR) for seq
            chunks 2*sc2 and 2*sc2+1 into one [128, 512] PSUM tile each
            (single bank-zeroing start covers both column halves), rope +
            normalize k, accumulate both pairs' KV for both chunks."""
            sca, scb = 2 * sc2, 2 * sc2 + 1
            kp = pv_ps.tile([P, 2, 256], f32, tag="pvps", name="kp")
            for ci, sc in enumerate((sca, scb)):
                for j in range(KC // 2):
                    nc.tensor.matmul(
                        kp[:, ci, :],
                        x8_sb[:, 2 * j:2 * j + 2, sc * P:(sc + 1) * P],
                        wk8_sb[:, 2 * j * 256:(2 * j + 2) * 256]
                        .rearrange("p (two m) -> p two m", two=2),
                        start=(ci == 0 and j == 0), stop=False,
                        perf_mode=DR, skip_group_check=True)
            vp = pv_ps.tile([P, 2, 256], f32, tag="pvps", name="vp")
            for ci, sc in enumerate((sca, scb)):
                for j in range(KC // 2):
                    nc.tensor.matmul(
                        vp[:, ci, :],
                        x8_sb[:, 2 * j:2 * j + 2, sc * P:(sc + 1) * P],
                        wv8_sb[:, 2 * j * 256:(2 * j + 2) * 256]
                        .rearrange("p (two m) -> p two m", two=2),
                        start=(ci == 0 and j == 0), stop=False,
                        perf_mode=DR, skip_group_check=True)
                for j in range(KC // 2):
                    nc.tensor.matmul(
                        vp[:, ci, :],
                        r8_sb[:, 2 * j:2 * j + 2, sc * P:(sc + 1) * P],
                        wv8s_sb[:, 2 * j * 256:(2 * j + 2) * 256]
                        .rearrange("p (two m) -> p two m", two=2),
                        start=False, stop=(ci == 1 and j == KC // 2 - 1),
                        perf_mode=DR, skip_group_check=True)
            # SCALE/16 rides on the v side (vp holds 16*v; k_hat is
            # normalization-invariant so KV picks up exactly SCALE)
            if sc2 % 2 == 0:
                nc.vector.tensor_scalar(
                    v_sb[:, sca:sca + 2, :, 0:64],
                    vp.rearrange("p c (h d) -> p c h d", h=4),
                    SCALE / 16.0, None, mybir.AluOpType.mult)
            else:
                nc.scalar.activation(
                    v_sb[:, sca:sca + 2, :, 0:64],
                    vp.rearrange("p c (h d) -> p c h d", h=4),
                    Copy, scale=SCALE / 16.0)
            # k rope in the free dim; stage kp to SBUF on ACT (undo x16)
            kraw = krope.tile([P, 2, 4, HD], bf16, tag="kraw", name="kraw")
            nc.scalar.activation(kraw.rearrange("p c h d -> p (c h d)"),
                                 kp.rearrange("p c m -> p (c m)"), Copy,
                                 scale=1.0 / W8SCALE)
            m2 = krope.tile([P, 2, 4, HD], bf16, tag="km2", name="km2")
            in0b, in1b = broadcast_tensor_aps(
                kraw[:, :, :, :], sts_sb[:, sca:sca + 2, :].rearrange(
                    "p c (o d) -> p c o d", o=1))
            nc.gpsimd.tensor_tensor(m2, in0b, in1b, mybir.AluOpType.mult)
            m1 = krope.tile([P, 2, 4, HD], bf16, tag="km1", name="km1")
            in0c, in1c = broadcast_tensor_aps(
                kraw[:, :, :, :], cts_sb[:, sca:sca + 2, :].rearrange(
                    "p c (o d) -> p c o d", o=1))
            nc.vector.tensor_tensor(m1, in0c, in1c, mybir.AluOpType.mult)
            rot = krope.tile([P, 2, 4, 32, 2], bf16, tag="krot", name="krot")
            m1v = m1.rearrange("p c h (d two) -> p c h d two", two=2)
            m2v = m2.rearrange("p c h (d two) -> p c h d two", two=2)
            # rot_even = m1_even + m2_odd ; rot_odd = m1_odd + m2_even
            nc.vector.tensor_add(rot[:, :, :, :, 0], m1v[:, :, :, :, 0],
                                 m2v[:, :, :, :, 1])
            nc.vector.tensor_add(rot[:, :, :, :, 1], m1v[:, :, :, :, 1],
                                 m2v[:, :, :, :, 0])
            sq = krope.tile([P, 512], bf16, tag="ksq", name="ksq")
            rotf = rot.rearrange("p c h d two -> p (c h d two)")
            nc.gpsimd.tensor_mul(sq, rotf, rotf)
            ssq = krope.tile([P, 8], bf16, tag="kssq", name="kssq")
            with nc.allow_low_precision(reason="k ssq ~64, bf16 adds 0.2% to "
                                        "an 0.4%-noise quantity"):
                nc.vector.tensor_reduce(
                    ssq, sq.rearrange("p (g d) -> p g d", g=8),
                    mybir.AxisListType.X, mybir.AluOpType.add)
            rsq = krope.tile([P, 8, 1], bf16, tag="krsq", name="krsq")
            nc.scalar.activation(rsq.rearrange("p g o -> p (g o)"), ssq,
                                 AbsRsqrt)
            in0n, in1n = broadcast_tensor_aps(
                rot.rearrange("p c h d two -> p (c h) (d two)"),
                rsq[:, :, :])
            eng = nc.vector if sc2 % 2 == 0 else nc.gpsimd
            eng.tensor_tensor(
                kh_sb[:, sca:sca + 2, :, :].rearrange("p c h d -> p (c h) d"),
                in0n, in1n, mybir.AluOpType.mult)
            for sc in (sca, scb):
                for pi in range(2):
                    nc.tensor.matmul(
                        kv_ps[pi],
                        kh_sb[:, sc, 2 * pi:2 * pi + 2, :].rearrange(
                            "p h d -> p (h d)"),
                        v_sb[:, sc, 2 * pi:2 * pi + 2, :].rearrange(
                            "p h e -> p (h e)"),
                        start=(sc == 0), stop=False,
                        skip_group_check=True)

        def kv_finish(pi):
            # pass 1: KV to SBUF; pull sum_k_hat rows out of columns 64 /
            # 129 via PE transpose, scale by -1/S, then rank-1 update
            # KV'' = KV - sk (x) svrow/S (svrow's 65th entry is S, which
            # zeroes column 64); pass 2: corrected KV back to SBUF.
            nc.vector.tensor_copy(kv_sb[pi], kv_ps[pi])
            for j, (pr, cr) in enumerate(((slice(0, 64), 64),
                                          (slice(64, 128), 129))):
                i = 2 * pi + j
                tp = po_ps.tile([1, 64], bf16, tag="pops", name="tp")
                nc.tensor.matmul(tp, kv_sb[pi][pr, cr:cr + 1],
                                 id64_sb[pr, :], is_transpose=True)
                nc.vector.tensor_scalar(skm_sb[:, i, :], tp, -1.0 / S, None,
                                        mybir.AluOpType.mult)
            nc.tensor.matmul(
                kv_ps[pi][0:64, 0:65], skm_sb[:, 2 * pi, :],
                svrow_sb[:, 2 * pi, :],
                start=False, stop=False, skip_group_check=True)
            nc.tensor.matmul(
                kv_ps[pi][64:128, 65:130], skm_sb[:, 2 * pi + 1, :],
                svrow_sb[:, 2 * pi + 1, :],
                start=False, stop=True, skip_group_check=True,
                tile_position=(0, 64))
            nc.vector.tensor_copy(kv_sb[pi], kv_ps[pi])

        def attn_qq(pi, qq):
            qqs = slice(qq * SL, (qq + 1) * SL)
            pool = po_ps if (pi + qq) % 2 == 0 else pv_ps
            tagn = "pops" if (pi + qq) % 2 == 0 else "pvps"
            o_a = pool.tile([65, SL], f32, tag=tagn, name="o_a")
            o_b = pool.tile([65, SL], f32, tag=tagn, name="o_b")
            nc.tensor.matmul(o_a, kv_sb[pi][0:64, 0:65],
                             qhat[pi][0:64, qqs], start=True, stop=True)
            nc.tensor.matmul(o_b, kv_sb[pi][64:128, 65:130],
                             qhat[pi][64:128, qqs], start=True, stop=True,
                             tile_position=(64, 0))
            # ao = (o + sum_v) / S: head A fused on DVE, head B on ACT
            # (Identity with per-partition bias = sum_v/S); B staged then
            # DMA'd to partitions 64:128
            nc.vector.tensor_scalar(ao[pi][0:64, qqs], o_a[0:64, :],
                                    sv_sb[:, 2 * pi:2 * pi + 1], 1.0 / S,
                                    mybir.AluOpType.add, mybir.AluOpType.mult)
            tm_b = bc_pool.tile([64, SL], bf16, tag="tmb", name="tm_b")
            nc.scalar.activation(tm_b, o_b[0:64, :], Identity,
                                 bias=svs_sb[:, 2 * pi + 1:2 * pi + 2],
                                 scale=1.0 / S)
            nc.sync.dma_start(out=ao[pi][64:128, qqs], in_=tm_b)

        def outproj_qq(qq):
            for sc in range(4 * qq, 4 * qq + 4):
                stg = out_stage.tile([P, 2 * SL], bf16, tag="ostg", name="stg")
                for osl in range(2):
                    if osl == 0:
                        op = kv_ps_pool.tile([P, SL], f32, tag="kvps",
                                             name="op")
                    else:
                        op = psA.tile([P, SL], f32, tag="psA", name="op")
                    nc.tensor.matmul(
                        op, ao[0][:, sc * P:(sc + 1) * P],
                        wout_sb[:, osl * SL:(osl + 1) * SL],
                        start=True, stop=False)
                    nc.tensor.matmul(
                        op, ao[1][:, sc * P:(sc + 1) * P],
                        wout_sb[:, D + osl * SL:D + (osl + 1) * SL],
                        start=False, stop=True)
                    if osl == 0:
                        nc.scalar.activation(stg[:, 0:SL], op, Copy)
                    else:
                        nc.vector.tensor_copy(stg[:, SL:2 * SL], op)
                nc.sync.dma_start(out=part[sc * P:(sc + 1) * P, :], in_=stg)

        # ---- emission order = scheduling priority ----
        for ql in range(NSL):
            q_slice(0, ql)
            kv_chunk2(2 * ql)
            q_slice(1, ql)
            kv_chunk2(2 * ql + 1)
        kv_finish(0)
        attn_qq(0, 0)
        kv_finish(1)
        attn_qq(1, 0)
        attn_qq(0, 1)
        outproj_qq(0)
        attn_qq(1, 1)
        attn_qq(0, 2)
        outproj_qq(1)
        attn_qq(1, 2)
        attn_qq(0, 3)
        outproj_qq(2)
        attn_qq(1, 3)
        outproj_qq(3)


def _host_prep(tokens, qkv_w, qkv_b, out_w):
    """Build the 8 per-core input maps."""
    CT, STp, CTs, STs, pswap, bb, id64 = _consts()
    in_maps = []
    for core in range(NCORES):
        b = core // 4
        g = core % 4
        heads = [4 * g + i for i in range(4)]
        xTf = np.ascontiguousarray(tokens[b].T)
        xkc = np.ascontiguousarray(
            xTf.reshape(KC, P, S).transpose(1, 0, 2).reshape(P, KC * S))
        x8 = xkc.astype(ml_dtypes.float8_e4m3)
        r8 = (16.0 * (xkc - x8.astype(np.float32))).astype(
            ml_dtypes.float8_e4m3)

        def wq_tile(pair):
            rows = np.r_[heads[2 * pair] * HD:heads[2 * pair] * HD + HD,
                         heads[2 * pair + 1] * HD:
                         heads[2 * pair + 1] * HD + HD]
            Wt = qkv_w[rows] * W8SCALE                               # [128, D]
            return np.ascontiguousarray(Wt.T).reshape(KC, P, P).transpose(1, 0, 2).reshape(P, KC * P)

        wq8 = np.ascontiguousarray(
            np.concatenate([wq_tile(0), wq_tile(1)], axis=1)).astype(
                ml_dtypes.float8_e4m3)

        def vstyle(base, scale):
            rows = np.r_[tuple(np.arange(base + h * HD, base + (h + 1) * HD)
                               for h in heads)]
            WT = np.ascontiguousarray(qkv_w[rows].T) * scale         # [D, 256]
            return WT.reshape(KC, P, 256).transpose(1, 0, 2).reshape(
                P, KC * 256)

        wk8 = vstyle(D, W8SCALE).astype(ml_dtypes.float8_e4m3)
        wv8 = vstyle(2 * D, 16.0).astype(ml_dtypes.float8_e4m3)
        wv8s = vstyle(2 * D, 1.0).astype(ml_dtypes.float8_e4m3)

        wout_blocks = []
        for pair in range(2):
            wcols = np.r_[tuple(np.arange(h * HD, (h + 1) * HD)
                                for h in heads[2 * pair:2 * pair + 2])]
            wout_blocks.append(np.ascontiguousarray(out_w[:, wcols].T))  # [128, D]
        wout_h = np.ascontiguousarray(
            np.concatenate(wout_blocks, axis=1)).astype(ml_dtypes.bfloat16)

        # host-exact sum_k v per head: (sum_s x) @ Wv^T in f64
        xsum = tokens[b].astype(np.float64).sum(axis=0)              # [D]
        sv = np.empty((64, 4), dtype=np.float32)
        for i, h in enumerate(heads):
            Wvh = qkv_w[2 * D + h * HD:2 * D + (h + 1) * HD].astype(np.float64)
            sv[:, i] = (Wvh @ xsum).astype(np.float32)
        svs = (sv / np.float32(S)).astype(np.float32)
        svrow = np.zeros((1, 4 * 65), dtype=np.float32)
        for i in range(4):
            svrow[0, i * 65:i * 65 + 64] = sv[:, i]
            svrow[0, i * 65 + 64] = float(S)

        in_maps.append({
            "x8": x8, "r8": r8, "wq8": wq8, "wk8": np.ascontiguousarray(wk8),
            "wv8": np.ascontiguousarray(wv8),
            "wv8s": np.ascontiguousarray(wv8s), "wout": wout_h,
            "ct": CT, "st": STp, "cts": CTs, "sts": STs,
            "pswap": pswap, "bb": bb, "id64": id64, "sv": sv, "svs": svs,
            "svrow": svrow.astype(ml_dtypes.bfloat16),
        })
    return in_maps


def kernel(tokens, qkv_w, qkv_b, out_w, out_b, _trace=False, _tmpdir=None):
    tokens = np.asarray(tokens, dtype=np.float32)
    qkv_w = np.asarray(qkv_w, dtype=np.float32)
    qkv_b = np.asarray(qkv_b, dtype=np.float32)
    out_w = np.asarray(out_w, dtype=np.float32)
    out_b = np.asarray(out_b, dtype=np.float32)

    if np.any(qkv_b):
        raise NotImplementedError(
            "kernel compiled for qkv_b == 0 (spec fill: zeros)")
    if "nc" not in _CACHE:
        _CACHE["nc"] = _build()
    nc = _CACHE["nc"]

    in_maps = _host_prep(tokens, qkv_w, qkv_b, out_w)
    res = run_bass_kernel_spmd(nc, in_maps, list(range(NCORES)),
                               trace=_trace, tmpdir=_tmpdir)
    out = np.zeros((B, S, D), dtype=np.float32)
    for core in range(NCORES):
        out[core // 4] += res.results[core]["part"].astype(np.float32)
    out += out_b[None, None, :]
    if _trace:
        return out, res
    return out
